# revision 9
# baseline (speedup 1.0000x reference)
"""LoRA linear (dropout -> x @ A.T @ B.T * scaling) on 8 TRN2 NeuronCores.

Data-parallel over tokens: each core handles T/8 = 2048 tokens; lora_A/lora_B
are replicated. All math in fp32.

Per-core pipeline, per 128-token tile:
  DMA x,u -> DVE mask=(u>=p), xd=x*mask -> PE-transpose xd (32x [128,128])
  -> ACT copy PSUM->SBUF -> matmul1 hT[64,128] (accum over 32 K-chunks)
  -> matmul2 out[128,512]x8 -> ACT copy -> DMA out.
The dropout 1/(1-p) and the LoRA alpha/r scaling are folded into lora_B on
the host, so no extra scaling pass is needed on-chip.
"""

import sys

sys.path.insert(0, "/opt/trn_rl_repo")

import numpy as np

import concourse.bacc as bacc
import concourse.bass as bass
import concourse.tile as tile
from concourse import masks, mybir
from concourse.bass_utils import run_bass_kernel_spmd

N_CORES = 8
T, IN, OUT, R = 16384, 4096, 4096, 64
TS = T // N_CORES  # tokens per core
P_DROP = 0.1
SCALE = (128.0 / 64.0) / (1.0 - P_DROP)  # alpha/r * 1/(1-p), folded into B

F32 = mybir.dt.float32
KC = IN // 128  # contraction chunks (32)
NOC = OUT // 512  # output column chunks (8)


def _emit(tc, x, u, a, b, o, ts):
    """Emit the per-core program. ts = tokens this core (multiple of 128)."""
    nc = tc.nc
    ntt = ts // 128
    from contextlib import ExitStack

    with ExitStack() as ctx:
        const = ctx.enter_context(tc.tile_pool(name="const", bufs=1))
        setup = ctx.enter_context(tc.tile_pool(name="setup", bufs=1))
        xpool = ctx.enter_context(tc.tile_pool(name="xp", bufs=2))
        upool = ctx.enter_context(tc.tile_pool(name="up", bufs=2))
        xtpool = ctx.enter_context(tc.tile_pool(name="xtp", bufs=1))
        hpool = ctx.enter_context(tc.tile_pool(name="hp", bufs=2))
        opool = ctx.enter_context(tc.tile_pool(name="op", bufs=2))
        pst = ctx.enter_context(tc.tile_pool(name="pst", bufs=3, space="PSUM"))
        psh = ctx.enter_context(tc.tile_pool(name="psh", bufs=2, space="PSUM"))
        pso = ctx.enter_context(tc.tile_pool(name="pso", bufs=2, space="PSUM"))

        ident = const.tile([128, 128], F32)
        masks.make_identity(nc, ident[:])

        # aT[:, kc*64:(kc+1)*64] = A[:, kc*128:(kc+1)*128].T   ([128 i, 64 r])
        a_nat = setup.tile([R, IN], F32, tag="setup")
        nc.sync.dma_start(a_nat[:], a[:, :])
        aT = const.tile([128, KC * R], F32)
        for kc in range(KC):
            tp = pst.tile([128, 512], F32, tag="tp")
            nc.tensor.transpose(
                tp[:, :R], a_nat[:, kc * 128 : (kc + 1) * 128], ident[:R, :R]
            )
            nc.scalar.copy(aT[:, kc * R : (kc + 1) * R], tp[:, :R])

        # bT[:, oc*128:(oc+1)*128] = B[oc*128:(oc+1)*128, :].T  ([64 r, 128 o])
        nbc = OUT // 128
        b_sb = setup.tile([128, nbc * R], F32, tag="setup")
        nc.sync.dma_start(
            b_sb[:].rearrange("p (c r) -> p c r", c=nbc),
            b.rearrange("(c p) r -> p c r", p=128),
        )
        bT = const.tile([R, OUT], F32)
        for oc in range(nbc):
            tp = pst.tile([128, 512], F32, tag="tp")
            nc.tensor.transpose(
                tp[:R, :128], b_sb[:, oc * R : (oc + 1) * R], ident[:, :]
            )
            nc.scalar.copy(bT[:, oc * 128 : (oc + 1) * 128], tp[:R, :128])

        # Process groups of 4 token tiles (512 tokens) so mm1 runs at N=512:
        # 32 accumulating matmuls per group instead of 128 N=128 ones — cuts
        # per-instruction PE overhead, the second-biggest PE cost after the
        # (irreducible fp32) transposes.
        ngrp = ntt // 4
        GT = 512  # tokens per group
        for gi in range(ngrp):
            # xdT_g chunk kc holds xd[group_rows, kc*128:(kc+1)*128].T for
            # all 512 tokens: layout [128 i, kc(32) * t(512)].
            xdT_g = xtpool.tile([128, KC * GT], F32)
            xdT_v = xdT_g[:].rearrange("p (kc t) -> p kc t", kc=KC)
            for j in range(4):
                rows = slice(gi * GT + j * 128, gi * GT + (j + 1) * 128)
                xt = xpool.tile([128, IN], F32)
                nc.sync.dma_start(xt[:], x[rows, :])
                ut = upool.tile([128, IN], F32)
                nc.sync.dma_start(ut[:], u[rows, :])

                # dropout: ut <- (ut >= p), xt <- xt * ut
                nc.vector.tensor_scalar(
                    ut[:], ut[:], P_DROP, None, mybir.AluOpType.is_ge
                )
                nc.vector.tensor_tensor(
                    xt[:], xt[:], ut[:], mybir.AluOpType.mult
                )

                for g in range(IN // 512):
                    tp = pst.tile([128, 512], F32, tag="tp")
                    for jj in range(4):
                        kc = g * 4 + jj
                        nc.tensor.transpose(
                            tp[:, jj * 128 : (jj + 1) * 128],
                            xt[:, kc * 128 : (kc + 1) * 128],
                            ident[:],
                        )
                    # scatter the 4 chunks to column block j of each kc row
                    nc.scalar.copy(
                        xdT_v[:, g * 4 : (g + 1) * 4, j * 128 : (j + 1) * 128],
                        tp[:].rearrange("p (c t) -> p c t", c=4),
                    )

            # hT[64, 512] = sum_kc aT_kc.T @ xdT_kc
            ph = psh.tile([R, GT], F32)
            for kc in range(KC):
                nc.tensor.matmul(
                    ph[:],
                    aT[:, kc * R : (kc + 1) * R],
                    xdT_v[:, kc, :],
                    start=(kc == 0),
                    stop=(kc == KC - 1),
                )
            hT = hpool.tile([R, GT], F32)
            nc.vector.tensor_copy(hT[:], ph[:])

            # out[j*128:(j+1)*128, :] = hT[:, j-slice].T @ bT
            for j in range(4):
                rows = slice(gi * GT + j * 128, gi * GT + (j + 1) * 128)
                osb = opool.tile([128, OUT], F32)
                for oc in range(NOC):
                    po = pso.tile([128, 512], F32, tag="po")
                    nc.tensor.matmul(
                        po[:],
                        hT[:, j * 128 : (j + 1) * 128],
                        bT[:, oc * 512 : (oc + 1) * 512],
                        start=True,
                        stop=True,
                    )
                    nc.scalar.copy(osb[:, oc * 512 : (oc + 1) * 512], po[:])
                nc.sync.dma_start(o[rows, :], osb[:])


def build_nc(ts=TS):
    nc = bacc.Bacc()
    x_d = nc.declare_dram_parameter("x", [ts, IN], F32, isOutput=False)
    u_d = nc.declare_dram_parameter("u", [ts, IN], F32, isOutput=False)
    a_d = nc.declare_dram_parameter("a", [R, IN], F32, isOutput=False)
    b_d = nc.declare_dram_parameter("b", [OUT, R], F32, isOutput=False)
    o_d = nc.declare_dram_parameter("o", [ts, OUT], F32, isOutput=True)
    with tile.TileContext(nc) as tc:
        _emit(tc, x_d[:], u_d[:], a_d[:], b_d[:], o_d[:], ts)
    # run_bass_via_pjrt expects a finalized module; Bacc.finalize() also runs
    # the TRN2 sync-wait legalization (move_matmul_waits_to_ldweights etc.).
    if not nc.is_finalized():
        nc.finalize()
    return nc


_NC_CACHE = None


def _get_nc():
    global _NC_CACHE
    if _NC_CACHE is None:
        _NC_CACHE = build_nc()
    return _NC_CACHE


def _in_maps(x, lora_A, lora_B, drop_u):
    bs = np.ascontiguousarray(lora_B.astype(np.float32) * np.float32(SCALE))
    a = np.ascontiguousarray(lora_A.astype(np.float32))
    return [
        {
            "x": np.ascontiguousarray(x[c * TS : (c + 1) * TS]),
            "u": np.ascontiguousarray(drop_u[c * TS : (c + 1) * TS]),
            "a": a,
            "b": bs,
        }
        for c in range(N_CORES)
    ]


def run_spmd(x, lora_A, lora_B, drop_u, **kw):
    res = run_bass_kernel_spmd(
        _get_nc(), _in_maps(x, lora_A, lora_B, drop_u), list(range(N_CORES)), **kw
    )
    out = np.concatenate([r["o"] for r in res.results], axis=0)
    return out, res


def kernel(x, lora_A, lora_B, drop_u):
    out, _ = run_spmd(x, lora_A, lora_B, drop_u)
    return out


# revision 11
# speedup vs baseline: 1.1665x; 1.1665x over previous
"""LoRA linear (dropout -> x @ A.T @ B.T * scaling) on 8 TRN2 NeuronCores.

Data-parallel over tokens: each core handles T/8 = 2048 tokens; lora_A/lora_B
are replicated. All math in fp32.

Per-core pipeline, per 128-token tile:
  DMA x,u -> DVE mask=(u>=p), xd=x*mask -> PE-transpose xd (32x [128,128])
  -> ACT copy PSUM->SBUF -> matmul1 hT[64,128] (accum over 32 K-chunks)
  -> matmul2 out[128,512]x8 -> ACT copy -> DMA out.
The dropout 1/(1-p) and the LoRA alpha/r scaling are folded into lora_B on
the host, so no extra scaling pass is needed on-chip.
"""

import sys

sys.path.insert(0, "/opt/trn_rl_repo")

import numpy as np

import concourse.bacc as bacc
import concourse.bass as bass
import concourse.tile as tile
from concourse import masks, mybir
from concourse.bass_utils import run_bass_kernel_spmd

N_CORES = 8
T, IN, OUT, R = 16384, 4096, 4096, 64
TS = T // N_CORES  # tokens per core
P_DROP = 0.1
SCALE = (128.0 / 64.0) / (1.0 - P_DROP)  # alpha/r * 1/(1-p), folded into B

F32 = mybir.dt.float32
KC = IN // 128  # contraction chunks (32)
NOC = OUT // 512  # output column chunks (8)


def _emit(tc, x, u, a, b, o, ts):
    """Emit the per-core program. ts = tokens this core (multiple of 128)."""
    nc = tc.nc
    ntt = ts // 128
    from contextlib import ExitStack

    with ExitStack() as ctx:
        const = ctx.enter_context(tc.tile_pool(name="const", bufs=1))
        setup = ctx.enter_context(tc.tile_pool(name="setup", bufs=1))
        xpool = ctx.enter_context(tc.tile_pool(name="xp", bufs=2))
        upool = ctx.enter_context(tc.tile_pool(name="up", bufs=2))
        xtpool = ctx.enter_context(tc.tile_pool(name="xtp", bufs=2))
        hpool = ctx.enter_context(tc.tile_pool(name="hp", bufs=2))
        opool = ctx.enter_context(tc.tile_pool(name="op", bufs=2))
        pst = ctx.enter_context(tc.tile_pool(name="pst", bufs=3, space="PSUM"))
        psh = ctx.enter_context(tc.tile_pool(name="psh", bufs=2, space="PSUM"))
        pso = ctx.enter_context(tc.tile_pool(name="pso", bufs=2, space="PSUM"))

        ident = const.tile([128, 128], F32)
        masks.make_identity(nc, ident[:])

        # aT[:, kc*64:(kc+1)*64] = A[:, kc*128:(kc+1)*128].T   ([128 i, 64 r])
        a_nat = setup.tile([R, IN], F32, tag="setup")
        nc.sync.dma_start(a_nat[:], a[:, :])
        aT = const.tile([128, KC * R], F32)
        for kc in range(KC):
            tp = pst.tile([128, 512], F32, tag="tp")
            nc.tensor.transpose(
                tp[:, :R], a_nat[:, kc * 128 : (kc + 1) * 128], ident[:R, :R]
            )
            nc.scalar.copy(aT[:, kc * R : (kc + 1) * R], tp[:, :R])

        # bT[:, oc*128:(oc+1)*128] = B[oc*128:(oc+1)*128, :].T  ([64 r, 128 o])
        nbc = OUT // 128
        b_sb = setup.tile([128, nbc * R], F32, tag="setup")
        nc.sync.dma_start(
            b_sb[:].rearrange("p (c r) -> p c r", c=nbc),
            b.rearrange("(c p) r -> p c r", p=128),
        )
        bT = const.tile([R, OUT], F32)
        for oc in range(nbc):
            tp = pst.tile([128, 512], F32, tag="tp")
            nc.tensor.transpose(
                tp[:R, :128], b_sb[:, oc * R : (oc + 1) * R], ident[:, :]
            )
            nc.scalar.copy(bT[:, oc * 128 : (oc + 1) * 128], tp[:R, :128])

        for ti in range(ntt):
            rows = slice(ti * 128, (ti + 1) * 128)
            xt = xpool.tile([128, IN], F32)
            nc.sync.dma_start(xt[:], x[rows, :])
            ut = upool.tile([128, IN], F32)
            nc.sync.dma_start(ut[:], u[rows, :])

            # dropout: ut <- (ut >= p), xt <- xt * ut
            nc.vector.tensor_scalar(
                ut[:], ut[:], P_DROP, None, mybir.AluOpType.is_ge
            )
            nc.vector.tensor_tensor(xt[:], xt[:], ut[:], mybir.AluOpType.mult)

            # xdT[:, kc*128:(kc+1)*128] = xd[:, kc*128:(kc+1)*128].T
            xdT = xtpool.tile([128, IN], F32)
            for g in range(IN // 512):
                tp = pst.tile([128, 512], F32, tag="tp")
                for j in range(4):
                    kc = g * 4 + j
                    nc.tensor.transpose(
                        tp[:, j * 128 : (j + 1) * 128],
                        xt[:, kc * 128 : (kc + 1) * 128],
                        ident[:],
                    )
                nc.scalar.copy(xdT[:, g * 512 : (g + 1) * 512], tp[:])

            # hT[64, 128] = sum_kc aT_kc.T @ xdT_kc
            ph = psh.tile([R, 128], F32)
            for kc in range(KC):
                nc.tensor.matmul(
                    ph[:],
                    aT[:, kc * R : (kc + 1) * R],
                    xdT[:, kc * 128 : (kc + 1) * 128],
                    start=(kc == 0),
                    stop=(kc == KC - 1),
                )
            hT = hpool.tile([R, 128], F32)
            nc.vector.tensor_copy(hT[:], ph[:])

            # out[128, 512*8] = hT.T @ bT
            osb = opool.tile([128, OUT], F32)
            for oc in range(NOC):
                po = pso.tile([128, 512], F32, tag="po")
                nc.tensor.matmul(
                    po[:],
                    hT[:],
                    bT[:, oc * 512 : (oc + 1) * 512],
                    start=True,
                    stop=True,
                )
                nc.scalar.copy(osb[:, oc * 512 : (oc + 1) * 512], po[:])
            nc.sync.dma_start(o[rows, :], osb[:])


def build_nc(ts=TS):
    nc = bacc.Bacc()
    x_d = nc.declare_dram_parameter("x", [ts, IN], F32, isOutput=False)
    u_d = nc.declare_dram_parameter("u", [ts, IN], F32, isOutput=False)
    a_d = nc.declare_dram_parameter("a", [R, IN], F32, isOutput=False)
    b_d = nc.declare_dram_parameter("b", [OUT, R], F32, isOutput=False)
    o_d = nc.declare_dram_parameter("o", [ts, OUT], F32, isOutput=True)
    with tile.TileContext(nc) as tc:
        _emit(tc, x_d[:], u_d[:], a_d[:], b_d[:], o_d[:], ts)
    # run_bass_via_pjrt expects a finalized module; Bacc.finalize() also runs
    # the TRN2 sync-wait legalization (move_matmul_waits_to_ldweights etc.).
    if not nc.is_finalized():
        nc.finalize()
    return nc


_NC_CACHE = None


def _get_nc():
    global _NC_CACHE
    if _NC_CACHE is None:
        _NC_CACHE = build_nc()
    return _NC_CACHE


def _in_maps(x, lora_A, lora_B, drop_u):
    bs = np.ascontiguousarray(lora_B.astype(np.float32) * np.float32(SCALE))
    a = np.ascontiguousarray(lora_A.astype(np.float32))
    return [
        {
            "x": np.ascontiguousarray(x[c * TS : (c + 1) * TS]),
            "u": np.ascontiguousarray(drop_u[c * TS : (c + 1) * TS]),
            "a": a,
            "b": bs,
        }
        for c in range(N_CORES)
    ]


def run_spmd(x, lora_A, lora_B, drop_u, **kw):
    res = run_bass_kernel_spmd(
        _get_nc(), _in_maps(x, lora_A, lora_B, drop_u), list(range(N_CORES)), **kw
    )
    out = np.concatenate([r["o"] for r in res.results], axis=0)
    return out, res


def kernel(x, lora_A, lora_B, drop_u):
    out, _ = run_spmd(x, lora_A, lora_B, drop_u)
    return out
